# revision 1
# baseline (speedup 1.0000x reference)
"""Trainium2 Bass kernel for nn_AttentionBlock (ragged_sequence, 16 equal
segments of 2048 q/kv tokens, HID=256, QD=64) on 8 NeuronCores.

Sharding: 2 segments (4096 rows) per core, weights replicated, outputs
concatenated host-side (attention is block-diagonal per segment -> no
cross-core communication needed).

Design: fp8e4+DoubleRow matmuls for the Q/K/V projections and P@V
(P written as fp8 directly by the exp ACTIVATE; ones-column on V yields
the softmax denominator), softmax row-scale invariance removes the
reciprocal (x = den*q + att_unnormalized feeds LayerNorm unchanged),
and a 3-stage software pipeline interleaves, per 128-row i-tile: the
score matmuls + exp of chunk c, the AV+stats stage of chunk c-1, and
the fc/norm epilogue of chunk c-2 - so the PE, ACT, and DVE queues all
stay fed. kqq PSUM->SBUF casts run on the (otherwise idle) ACT engine
during phase 1; outputs are stored bf16 and upcast on the host.
"""

import os
import sys

os.environ.setdefault("MYCRO_LOCAL_CACHE", "1")
if "/opt/trn_rl_repo" not in sys.path:
    sys.path.insert(0, "/opt/trn_rl_repo")

import numpy as np

HID = 256
QD = 64
LQ = 2048
LH = 2048
B = 16
NCORES = 8
SEGS = 2                  # segments per core
ROWS = SEGS * LQ          # 4096 q rows per core
EPS = 1e-5
SCALE = 1.0 / 8.0         # 1/sqrt(QD)
LDWOPT = os.environ.get("BASS_LDWOPT") == "1"

_built = {}               # (apply0,) -> nc


def _patch_act_tables():
    """Make the act-table pass choose the combined exp+ln table for every
    activation: blank all other tables (indices preserved so walrus's
    act_func_set_id remap stays correct). Avoids 100+ ACT_TABLE_LOADs
    (1.28us each) from alternating Exp/Ln table picks."""
    import functools
    import concourse.hw_specs as hw_specs
    import concourse.bacc as bacc_mod
    if getattr(hw_specs, "_attn_tables_patched", False):
        return
    orig = hw_specs.get_activation_tables

    @functools.cache
    def patched(arch):
        tabs = dict(orig(arch))
        joint = "natural_log_exp_and_others"
        assert joint in tabs, sorted(tabs)
        return {name: (funcs if name == joint else set())
                for name, funcs in tabs.items()}

    hw_specs.get_activation_tables = patched
    bacc_mod.get_activation_tables = patched
    hw_specs._attn_tables_patched = True


def _patch_ldwopt():
    from concourse import bass_utils as _bu
    if getattr(_bu, "_ldwopt_patched", False):
        return
    _orig = _bu.run_command

    def _rc(cmd, **kw):
        cmd = ["--enable-ldw-opt=true" if c == "--enable-ldw-opt=false"
               else c for c in cmd]
        return _orig(cmd, **kw)

    _bu.run_command = _rc
    _bu._ldwopt_patched = True


def _build(apply0: bool):
    from concourse import bacc, bass, mybir, tile

    _patch_act_tables()
    if LDWOPT:
        _patch_ldwopt()

    dt = mybir.dt
    f32 = dt.float32
    bf16 = dt.bfloat16
    f8 = dt.float8e4
    AF = mybir.ActivationFunctionType
    Alu = mybir.AluOpType
    DR = mybir.MatmulPerfMode.DoubleRow

    nc = bacc.Bacc("TRN2", target_bir_lowering=False, debug=False,
                   enable_asserts=False)

    qT8_d = nc.dram_tensor("qT8", [HID, ROWS], f8, kind="ExternalInput")
    hT8_d = nc.dram_tensor("hT8", [HID, ROWS], f8, kind="ExternalInput")
    q_d = nc.dram_tensor("q", [ROWS, HID], f32, kind="ExternalInput")
    wq8_d = nc.dram_tensor("WQ8", [128, 2 * QD], f8, kind="ExternalInput")
    wk8_d = nc.dram_tensor("WK8", [128, 2 * QD], f8, kind="ExternalInput")
    wv8_d = nc.dram_tensor("WV8", [128, 2 * HID], f8, kind="ExternalInput")
    fwT_d = nc.dram_tensor("FCWT", [HID, HID], bf16, kind="ExternalInput")
    fb_d = nc.dram_tensor("FCB", [1, HID], bf16, kind="ExternalInput")
    idt_d = nc.dram_tensor("IDT", [128, 128], bf16, kind="ExternalInput")
    if apply0:
        n0w_d = nc.dram_tensor("N0W", [128, HID], f32, kind="ExternalInput")
        n0b_d = nc.dram_tensor("N0B", [128, HID], f32, kind="ExternalInput")
    out_d = nc.dram_tensor("out", [ROWS, HID], bf16, kind="ExternalOutput")

    q_a = q_d.ap()
    out_a = out_d.ap()

    NJT = LH // 128           # 16 j-tiles per segment
    NIC = 2                   # 1024-col i-chunks per segment for scores
    ICW = LQ // NIC           # 1024
    NIL = ICW // 128          # 8 i-tiles per chunk
    VW = HID + 1              # V block width incl ones column

    with tile.TileContext(nc) as tc:
        with (
            tc.tile_pool(name="const", bufs=1) as cpool,
            tc.tile_pool(name="kqq", bufs=1) as kqq_pool,
            tc.tile_pool(name="vsb", bufs=1) as v_pool,
        ):
            # ---- constants ----
            wq_sb = cpool.tile([128, 2 * QD], f8)
            wk_sb = cpool.tile([128, 2 * QD], f8)
            wv_sb = cpool.tile([128, 2 * HID], f8)
            fw_sb = cpool.tile([128, 2 * HID], bf16)    # fc_w.T chunks
            fb_sb = cpool.tile([1, HID], bf16)
            one_sb = cpool.tile([1, 128], bf16)
            idt_sb = cpool.tile([128, 128], bf16)
            nc.sync.dma_start(wq_sb[:], wq8_d.ap()[:, :])
            nc.sync.dma_start(wk_sb[:], wk8_d.ap()[:, :])
            nc.sync.dma_start(wv_sb[:], wv8_d.ap()[:, :])
            for e in range(2):
                nc.sync.dma_start(fw_sb[:, e * HID:(e + 1) * HID],
                                  fwT_d.ap()[e * 128:(e + 1) * 128, :])
            nc.sync.dma_start(fb_sb[:], fb_d.ap()[:, :])
            nc.sync.dma_start(idt_sb[:], idt_d.ap()[:, :])
            nc.vector.memset(one_sb[:], 1.0)
            eps_sb = cpool.tile([128, 1], f32)
            nc.vector.memset(eps_sb[:], EPS)
            nb3_sb = cpool.tile([128, 1], f32)
            nc.vector.memset(nb3_sb[:], -3.0)
            cachebust = cpool.tile([1, 1], f32)
            nc.vector.memset(cachebust[:], 3.0 if LDWOPT else 2.0)
            if apply0:
                n0w_sb = cpool.tile([128, HID], f32)
                n0b_sb = cpool.tile([128, HID], f32)
                nc.sync.dma_start(n0w_sb[:], n0w_d.ap()[:, :])
                nc.sync.dma_start(n0b_sb[:], n0b_d.ap()[:, :])

            # persistent activations
            kT_sb = kqq_pool.tile([64, ROWS], bf16)     # K^T  [c, j_global]
            qq_sb = kqq_pool.tile([64, ROWS], bf16)     # qq^T [c, i_global]
            v_sb = v_pool.tile([128, SEGS * NJT * VW], f8)

            # ---------------- phase 1: projections (fp8 DoubleRow) --------
            with (
                tc.tile_pool(name="qhT", bufs=1) as qh_pool,
                tc.tile_pool(name="pp_kq", bufs=4,
                             space=bass.MemorySpace.PSUM) as pp_kq,
                tc.tile_pool(name="pp_v", bufs=4,
                             space=bass.MemorySpace.PSUM) as pp_v,
            ):
                q8t = qh_pool.tile([128, 2 * ROWS], f8, tag="q8")
                h8t = qh_pool.tile([128, 2 * ROWS], f8, tag="h8")
                # 2KB-per-partition chunks: enough DMA-queue parallelism
                # to hide the transfer without paying 600ns of sync-queue
                # issue cost per descriptor 32 times over
                for c in range(0, ROWS, 2048):
                    for e in range(2):
                        nc.sync.dma_start(
                            q8t[:, e * ROWS + c:e * ROWS + c + 2048],
                            qT8_d.ap()[e * 128:(e + 1) * 128, c:c + 2048])
                        nc.sync.dma_start(
                            h8t[:, e * ROWS + c:e * ROWS + c + 2048],
                            hT8_d.ap()[e * 128:(e + 1) * 128, c:c + 2048])
                q8r = q8t[:].rearrange("p (e c) -> p e c", e=2)
                h8r = h8t[:].rearrange("p (e c) -> p e c", e=2)
                wqr = wq_sb[:].rearrange("p (e m) -> p e m", e=2)
                wkr = wk_sb[:].rearrange("p (e m) -> p e m", e=2)
                wvr = wv_sb[:].rearrange("p (e m) -> p e m", e=2)

                # kT / qqT: [64, 512] chunks, one DoubleRow matmul each.
                # s0 columns first so segment-0 scores can start early.
                for half in range(2):
                    cols = range(half * LQ, (half + 1) * LQ, 512)
                    for dst, w_r, src in ((kT_sb, wkr, h8r),
                                          (qq_sb, wqr, q8r)):
                        for col in cols:
                            ps = pp_kq.tile([64, 512], f32, tag="kq")
                            nc.tensor.matmul(ps[:], w_r,
                                             src[:, :, col:col + 512],
                                             start=True, stop=True,
                                             perf_mode=DR)
                            nc.scalar.copy(dst[:, col:col + 512], ps[:])

                # V row-layout with ones column (fp8), 2 j-tiles per cast
                for s in range(SEGS):
                    for jt2 in range(NJT // 2):
                        ps = pp_v.tile([128, 2 * HID], f32, tag="v")
                        for u in range(2):
                            col = s * LH + (2 * jt2 + u) * 128
                            nc.tensor.matmul(ps[:, u * HID:(u + 1) * HID],
                                             h8r[:, :, col:col + 128],
                                             wvr, start=True, stop=True,
                                             perf_mode=DR)
                        base = (s * NJT + 2 * jt2) * VW
                        dst = v_sb[:, base:base + 2 * VW] \
                            .rearrange("p (two d) -> p two d", two=2)
                        nc.vector.tensor_copy(
                            dst[:, :, 0:HID],
                            ps[:].rearrange("p (two d) -> p two d", two=2))
                        for u in range(2):
                            nc.vector.memset(
                                v_sb[:, base + u * VW + HID:
                                     base + (u + 1) * VW], 1.0)

            # ---------------- phase 2: attention + epilogue ----------------
            with (
                tc.tile_pool(name="pt", bufs=2) as pt_pool,
                tc.tile_pool(name="qrow", bufs=10) as q_pool,
                tc.tile_pool(name="xs", bufs=3) as xs_pool,
                tc.tile_pool(name="ys", bufs=3) as ys_pool,
                tc.tile_pool(name="zt", bufs=4) as z_pool,
                tc.tile_pool(name="st8", bufs=3) as st8_pool,
                tc.tile_pool(name="outp", bufs=6) as o_pool,
                tc.tile_pool(name="ps_st", bufs=2,
                             space=bass.MemorySpace.PSUM) as ps_st,
                tc.tile_pool(name="ps_att", bufs=2,
                             space=bass.MemorySpace.PSUM) as ps_att,
                tc.tile_pool(name="ps_fc", bufs=1,
                             space=bass.MemorySpace.PSUM) as ps_fc,
                tc.tile_pool(name="ps_tp", bufs=1,
                             space=bass.MemorySpace.PSUM) as ps_tp,
            ):
                def emit_score_pair(ep, jp):
                    s, ic, pt = ep["s"], ep["ic"], ep["pt"]
                    icol = s * LQ + ic * ICW
                    for jt in (2 * jp, 2 * jp + 1):
                        st = ps_st.tile([128, ICW], f32, tag="st")
                        for h in range(2):
                            nc.tensor.matmul(
                                st[:, h * 512:(h + 1) * 512],
                                kT_sb[:, s * LH + jt * 128:
                                      s * LH + (jt + 1) * 128],
                                qq_sb[:, icol + h * 512:
                                      icol + (h + 1) * 512],
                                start=True, stop=True)
                        nc.scalar.activation(pt[:, jt * ICW:(jt + 1) * ICW],
                                             st[:], AF.Exp,
                                             scale=SCALE, bias=nb3_sb[:])

                def emit_a(ep, il):
                    """AV + x0s + row stats for one 128-row i-tile."""
                    s, ic, pt = ep["s"], ep["ic"], ep["pt"]
                    att = ps_att.tile([128, VW], f32, tag="att")
                    for jp in range(NJT // 2):
                        lhs = pt[:, jp * 2 * ICW:(jp + 1) * 2 * ICW] \
                            .rearrange("p (two i) -> p two i", two=2) \
                            [:, :, il * 128:(il + 1) * 128]
                        vb = (s * NJT + 2 * jp) * VW
                        rhs = v_sb[:, vb:vb + 2 * VW] \
                            .rearrange("p (two d) -> p two d", two=2)
                        nc.tensor.matmul(att[:], lhs, rhs,
                                         start=(jp == 0),
                                         stop=(jp == NJT // 2 - 1),
                                         perf_mode=DR)
                    row0 = s * LQ + (ic * NIL + il) * 128
                    if il % 2 == 0:
                        qt2 = q_pool.tile([128, 2 * HID], f32, tag="q")
                        nc.sync.dma_start(
                            qt2[:].rearrange("p (two d) -> p two d", two=2),
                            q_a[row0:row0 + 256, :]
                            .rearrange("(two p) d -> p two d", two=2))
                        ep["qt2"] = qt2
                    qt = ep["qt2"][:, (il % 2) * HID:(il % 2 + 1) * HID]
                    # x0s = den*q + att  (LN is row-scale invariant)
                    x0 = ep["xs"][:, il * HID:(il + 1) * HID]
                    nc.vector.scalar_tensor_tensor(
                        x0, qt, att[:, HID:HID + 1].opt(),
                        att[:, 0:HID], op0=Alu.mult, op1=Alu.add)
                    nc.vector.bn_stats(ep["mv6"][:, 6 * il:6 * il + 6], x0)
                    nc.vector.bn_aggr(ep["mva0"][:, 2 * il:2 * il + 2],
                                      ep["mv6"][:, 6 * il:6 * il + 6])

                def emit_mid(ep):
                    ln8a = st8_pool.tile([128, NIL], f32, tag="ln8a")
                    nc.scalar.activation(
                        ln8a[:].rearrange("p (t o) -> p t o", o=1),
                        ep["mva0"][:].rearrange("p (t o) -> p t o", o=2)
                        [:, :, 1:2],
                        AF.Ln, bias=eps_sb[:])
                    rstd8a = st8_pool.tile([128, NIL], f32, tag="r8a")
                    nc.scalar.activation(rstd8a[:], ln8a[:], AF.Exp,
                                         scale=-0.5)
                    ep["rstd8a"] = rstd8a

                def emit_b(ep, il):
                    xs_t, mva0, rstd8a = ep["xs"], ep["mva0"], ep["rstd8a"]
                    x0 = xs_t[:, il * HID:(il + 1) * HID]
                    z = z_pool.tile([128, HID], bf16, tag="z")
                    nc.vector.tensor_scalar(
                        z[:], x0, mva0[:, 2 * il:2 * il + 1].opt(),
                        rstd8a[:, il:il + 1].opt(),
                        op0=Alu.subtract, op1=Alu.mult)
                    if apply0:
                        z2 = z_pool.tile([128, HID], bf16, tag="z2")
                        nc.gpsimd.tensor_tensor(z2[:], z[:], n0w_sb[:],
                                                op=Alu.mult)
                        z3 = z_pool.tile([128, HID], bf16, tag="z3")
                        nc.gpsimd.tensor_tensor(z3[:], z2[:], n0b_sb[:],
                                                op=Alu.add)
                        zf = z3
                    else:
                        zf = z
                    hres = ps_fc.tile([128, HID], f32, tag="fc")
                    nc.tensor.matmul(hres[:], one_sb[:], fb_sb[:],
                                     start=True, stop=False)
                    tp = ps_tp.tile([128, 2 * 128], bf16, tag="tp")
                    for hh in range(2):
                        nc.tensor.transpose(
                            tp[:, hh * 128:(hh + 1) * 128],
                            zf[:, hh * 128:(hh + 1) * 128],
                            idt_sb[:])
                    zT = z_pool.tile([128, 2 * 128], bf16, tag="zT")
                    nc.vector.tensor_copy(zT[:], tp[:])
                    for hh in range(2):
                        nc.tensor.matmul(
                            hres[:], zT[:, hh * 128:(hh + 1) * 128],
                            fw_sb[:, hh * HID:(hh + 1) * HID],
                            start=False, stop=(hh == 1))
                    y0 = ep["ys"][:, il * HID:(il + 1) * HID]
                    nc.vector.scalar_tensor_tensor(
                        y0, hres[:], 0.0, zf[:],
                        op0=Alu.max, op1=Alu.add)
                    nc.vector.bn_stats(ep["mv6b"][:, 6 * il:6 * il + 6], y0)
                    nc.vector.bn_aggr(ep["mva1"][:, 2 * il:2 * il + 2],
                                      ep["mv6b"][:, 6 * il:6 * il + 6])

                def emit_end(ep, use_dve=False):
                    s, ic, mva1 = ep["s"], ep["ic"], ep["mva1"]
                    ln8b = st8_pool.tile([128, NIL], f32, tag="ln8b")
                    nc.scalar.activation(
                        ln8b[:].rearrange("p (t o) -> p t o", o=1),
                        mva1[:].rearrange("p (t o) -> p t o", o=2)[:, :, 1:2],
                        AF.Ln, bias=eps_sb[:])
                    rstd8b = st8_pool.tile([128, NIL], f32, tag="r8b")
                    nc.scalar.activation(rstd8b[:], ln8b[:], AF.Exp,
                                         scale=-0.5)
                    for il in range(NIL):
                        b1 = st8_pool.tile([128, 1], f32, tag="b1")
                        nc.vector.tensor_scalar(
                            b1[:], mva1[:, 2 * il:2 * il + 1],
                            rstd8b[:, il:il + 1].opt(), -1.0,
                            op0=Alu.mult, op1=Alu.mult)
                        if il % 2 == 0:
                            ot2 = o_pool.tile([128, 2 * HID], bf16,
                                              tag="ot")
                        if use_dve:
                            # tail: ACT is the serial chain after the last
                            # exp; DVE is idle there
                            nc.vector.tensor_scalar(
                                ot2[:, (il % 2) * HID:(il % 2 + 1) * HID],
                                ep["ys"][:, il * HID:(il + 1) * HID],
                                mva1[:, 2 * il:2 * il + 1].opt(),
                                rstd8b[:, il:il + 1].opt(),
                                op0=Alu.subtract, op1=Alu.mult)
                        else:
                            nc.scalar.activation(
                                ot2[:, (il % 2) * HID:(il % 2 + 1) * HID],
                                ep["ys"][:, il * HID:(il + 1) * HID],
                                AF.Identity, bias=b1[:],
                                scale=rstd8b[:, il:il + 1].opt())
                        if il % 2 == 1:
                            row0e = s * LQ + (ic * NIL + il - 1) * 128
                            nc.sync.dma_start(
                                out_a[row0e:row0e + 256, :]
                                .rearrange("(two p) d -> p two d", two=2),
                                ot2[:].rearrange("p (two d) -> p two d",
                                                 two=2))

                chunks = [(s, ic) for s in range(SEGS) for ic in range(NIC)]
                # 3-stage software pipeline: iteration ci emits, per i-tile
                # k: scores+exp of chunk ci, AV+x0s+stats of ci-1, and the
                # fc/norm epilogue of ci-2 — keeping PE/ACT/DVE queues all
                # fed at i-tile granularity.
                prev1 = prev2 = None
                for ci in range(len(chunks) + 2):
                    cur = None
                    if ci < len(chunks):
                        s, ic = chunks[ci]
                        cur = {
                            "s": s, "ic": ic,
                            "pt": pt_pool.tile([128, NJT * ICW], f8,
                                               tag="pt", name="pt"),
                            "xs": xs_pool.tile([128, NIL * HID], bf16,
                                               tag="xs", name="xs"),
                            "ys": ys_pool.tile([128, NIL * HID], bf16,
                                               tag="ys", name="ys"),
                            "mv6": st8_pool.tile([128, 6 * NIL], f32,
                                                 tag="mv6", name="mv6"),
                            "mva0": st8_pool.tile([128, 2 * NIL], f32,
                                                  tag="mva0", name="mva0"),
                            "mv6b": st8_pool.tile([128, 6 * NIL], f32,
                                                  tag="mv6b", name="mv6b"),
                            "mva1": st8_pool.tile([128, 2 * NIL], f32,
                                                  tag="mva1", name="mva1"),
                        }
                    for k in range(NIL):
                        # A/B stages first: their deps are chunks-old, so
                        # the PE can run them while ACT drains the previous
                        # slot's exps, instead of blocking behind the
                        # ps_st WAR of this slot's score matmuls.
                        if prev1 is not None:
                            emit_a(prev1, k)
                        if prev2 is not None:
                            emit_b(prev2, k)
                        if cur is not None:
                            emit_score_pair(cur, k)
                    if prev1 is not None:
                        emit_mid(prev1)
                    if prev2 is not None:
                        emit_end(prev2, use_dve=(ci >= len(chunks)))
                    prev1, prev2 = cur, prev1

    nc.compile()
    return nc


def _get_nc(apply0: bool):
    key = (bool(apply0),)
    if key not in _built:
        _built[key] = _build(apply0)
    return _built[key]


def _shard(inputs, apply0):
    from concourse import mybir
    bf = mybir.dt.np(mybir.dt.bfloat16)
    f8 = mybir.dt.np(mybir.dt.float8e4)

    q = np.ascontiguousarray(np.asarray(inputs["q"], dtype=np.float32))
    h = np.ascontiguousarray(np.asarray(inputs["h"], dtype=np.float32))
    WQ = np.asarray(inputs["WQ"], dtype=np.float32)
    WK = np.asarray(inputs["WK"], dtype=np.float32)
    WV = np.asarray(inputs["WV"], dtype=np.float32)
    fcw = np.asarray(inputs["fc_w"], dtype=np.float32)
    fcb = np.asarray(inputs["fc_b"], dtype=np.float32)

    def to8(x):
        return np.clip(x, -240.0, 240.0).astype(f8)

    def pack8(wT, m):
        # wT [HID, m] -> [128, 2, m] -> [128, 2*m] fp8 (e-chunks adjacent)
        return np.ascontiguousarray(
            wT.reshape(2, 128, m).transpose(1, 0, 2).reshape(128, 2 * m)
        ).astype(f8)

    WQ8 = pack8(np.ascontiguousarray(WQ.T), QD)
    WK8 = pack8(np.ascontiguousarray(WK.T), QD)
    WV8 = pack8(np.ascontiguousarray(WV.T), HID)
    FCWT = np.ascontiguousarray(fcw.T).astype(bf)
    FCB = np.ascontiguousarray(fcb.reshape(1, HID)).astype(bf)
    IDT = np.eye(128, dtype=np.float32).astype(bf)

    in_maps = []
    for c in range(NCORES):
        sl = slice(c * ROWS, (c + 1) * ROWS)
        m = {
            "qT8": to8(np.ascontiguousarray(q[sl].T)),
            "hT8": to8(np.ascontiguousarray(h[sl].T)),
            "q": q[sl],
            "WQ8": WQ8, "WK8": WK8, "WV8": WV8,
            "FCWT": FCWT, "FCB": FCB, "IDT": IDT,
        }
        if apply0:
            m["N0W"] = np.ascontiguousarray(
                np.broadcast_to(np.asarray(inputs["norm0_w"], np.float32),
                                (128, HID)))
            m["N0B"] = np.ascontiguousarray(
                np.broadcast_to(np.asarray(inputs["norm0_b"], np.float32),
                                (128, HID)))
        in_maps.append(m)
    return in_maps


def _run(inputs, trace=False, tmpdir=None):
    from concourse import bass_utils

    n0w = np.asarray(inputs["norm0_w"], np.float32)
    n0b = np.asarray(inputs["norm0_b"], np.float32)
    n1w = np.asarray(inputs["norm1_w"], np.float32)
    n1b = np.asarray(inputs["norm1_b"], np.float32)
    apply0 = not (np.allclose(n0w, 1.0) and np.allclose(n0b, 0.0))
    apply1 = not (np.allclose(n1w, 1.0) and np.allclose(n1b, 0.0))

    nc = _get_nc(apply0)
    in_maps = _shard(inputs, apply0)
    res = bass_utils.run_bass_kernel_spmd(
        nc, in_maps, core_ids=list(range(NCORES)), trace=trace,
        tmpdir=tmpdir)
    out = np.concatenate([np.asarray(res.results[c]["out"])
                          for c in range(NCORES)], axis=0).astype(np.float32)
    if apply1:
        out = out * n1w[None, :] + n1b[None, :]
    return out, res


def kernel(**inputs):
    out, _ = _run(inputs, trace=False)
    return out

